# revision 2
# baseline (speedup 1.0000x reference)
"""AF3-style pair attention (AttentionMix) on 8 TRN2 NeuronCores.

Sharding: data-parallel over leading pair dim b (384 rows -> 48/core).
bias[h,n,m] computed per-shard (bf16, pre-exponentiated), all-gathered
(Shared addr space), gathered into eb[j, t, h, n] via 3 strided DMAs.

v4: split+early collective, epilogue after attention core.\nv3 vs v2: software-pipelined phase-2 row loop (projections issued L rows
ahead; epilogue of row b-1 emitted during row b's attention so the serial
rec->wan->go->out chain overlaps the next row's QK/exp/em work); single
combined PSUM->SBUF copies (v, xT, bias, out); bf16 pair input.
"""
import sys, os
sys.path.insert(0, "/opt/trn_rl_repo")
import numpy as np
import ml_dtypes

N, C, H, D = 384, 128, 4, 32
NCORES, BL, T = 8, 48, 3
EPS = 1e-5
LOOK = 24

_cache = {}


def _build():
    import concourse.bass as bass
    import concourse.bacc as bacc
    import concourse.mybir as mybir
    import concourse.tile as tile

    f32 = mybir.dt.float32
    bf16 = mybir.dt.bfloat16
    AF = mybir.ActivationFunctionType
    ALU = mybir.AluOpType

    nc = bacc.Bacc("TRN2", target_bir_lowering=False, debug=False,
                   num_devices=NCORES)
    pair_l = nc.declare_dram_parameter("pair_l", [BL, N, C], bf16, isOutput=False)
    maskt_l = nc.declare_dram_parameter("maskt_l", [N, BL], f32, isOutput=False)
    wq = nc.declare_dram_parameter("wq", [C, C], bf16, isOutput=False)
    wk = nc.declare_dram_parameter("wk", [C, C], bf16, isOutput=False)
    wv = nc.declare_dram_parameter("wv", [C, C], bf16, isOutput=False)
    wg = nc.declare_dram_parameter("wg", [C, C], bf16, isOutput=False)
    wo = nc.declare_dram_parameter("wo", [C, C], bf16, isOutput=False)
    wb = nc.declare_dram_parameter("wb", [C, H], bf16, isOutput=False)
    id128 = nc.declare_dram_parameter("id128", [C, C], bf16, isOutput=False)
    ones32 = nc.declare_dram_parameter("ones32", [C, D], bf16, isOutput=False)
    out_p = nc.declare_dram_parameter("out", [BL, N, C], f32, isOutput=True)

    with tile.TileContext(nc) as tc:
        with (
            tc.tile_pool(name="const", bufs=1) as cp,
            tc.tile_pool(name="work", bufs=4) as wp,
            tc.tile_pool(name="proj", bufs=LOOK + 2) as pj,
            tc.tile_pool(name="att", bufs=3) as ap_,
            tc.tile_pool(name="epi", bufs=2) as epi,
            tc.tile_pool(name="ps_big", bufs=2, space="PSUM") as psb,
            tc.tile_pool(name="ps_acc", bufs=2, space="PSUM") as psa,
            tc.tile_pool(name="ps_tmp", bufs=2, space="PSUM") as pst,
            tc.tile_pool(name="dram", bufs=1, space="DRAM") as dp,
        ):
            # resident constants
            wq_s = cp.tile([C, C], bf16, tag="wq"); nc.sync.dma_start(wq_s[:], wq[:, :])
            wk_s = cp.tile([C, C], bf16, tag="wk"); nc.sync.dma_start(wk_s[:], wk[:, :])
            wv_s = cp.tile([C, C], bf16, tag="wv"); nc.sync.dma_start(wv_s[:], wv[:, :])
            wg_s = cp.tile([C, C], bf16, tag="wg"); nc.sync.dma_start(wg_s[:], wg[:, :])
            wo_s = cp.tile([C, C], bf16, tag="wo"); nc.sync.dma_start(wo_s[:], wo[:, :])
            wb_s = cp.tile([C, H], bf16, tag="wb"); nc.sync.dma_start(wb_s[:], wb[:, :])
            id_s = cp.tile([C, C], bf16, tag="id"); nc.sync.dma_start(id_s[:], id128[:, :])
            on_s = cp.tile([C, D], bf16, tag="on"); nc.sync.dma_start(on_s[:], ones32[:, :])
            eps_s = cp.tile([C, 1], f32, tag="eps")
            nc.vector.memset(eps_s[:], EPS)
            mk_s = cp.tile([C, T, BL], f32, tag="mk")
            nc.sync.dma_start(mk_s[:], maskt_l[:, :].rearrange("(t p) b -> p t b", p=C))

            xT = cp.tile([C, BL, N], bf16, tag="xT")
            bloc = cp.tile([C, T, H, BL], bf16, tag="bloc")

            # ---------------- phase 1: LN + transpose + local bias ----------
            HB = BL // 2
            bl_ds, bg_ds = [], []
            for b in range(BL):
                x = wp.tile([C, T, C], bf16, tag="x")
                nc.sync.dma_start(x[:], pair_l[b].rearrange("(t p) c -> p t c", p=C))
                st = wp.tile([C, T, 6], f32, tag="st")
                mv = wp.tile([C, T, 2], f32, tag="mv")
                for t in range(T):
                    nc.vector.bn_stats(st[:, t], x[:, t])
                    nc.vector.bn_aggr(mv[:, t], st[:, t])
                std = wp.tile([C, T], f32, tag="std")
                nc.scalar.activation(std[:], mv[:, :, 1], AF.Sqrt, bias=eps_s[:])
                rstd = wp.tile([C, T], f32, tag="rstd")
                nc.vector.reciprocal(rstd[:], std[:])
                nmr = wp.tile([C, T], f32, tag="nmr")
                nc.vector.scalar_tensor_tensor(
                    nmr[:], mv[:, :, 0], -1.0, rstd[:], ALU.mult, ALU.mult)
                xn = wp.tile([C, T, C], bf16, tag="xn")
                for t in range(T):
                    nc.scalar.activation(xn[:, t], x[:, t], AF.Identity,
                                         bias=nmr[:, t:t + 1],
                                         scale=rstd[:, t:t + 1])
                pt = pst.tile([C, T, C], bf16, tag="tmp")
                for t in range(T):
                    nc.tensor.matmul(pt[:, t], xn[:, t, :], id_s[:],
                                     is_transpose=True, skip_group_check=True)
                nc.vector.tensor_copy(
                    xT[:, b, :], pt[:].rearrange("p t c -> p (t c)"))
                pb = pst.tile([C, T, H], f32, tag="tmp")
                for t in range(T):
                    nc.tensor.matmul(pb[:, t], xT[:, b, t * C:(t + 1) * C],
                                     wb_s[:], start=True, stop=True,
                                     skip_group_check=True)
                nc.vector.tensor_copy(bloc[:, :, :, b], pb[:])

                if b == HB - 1 or b == BL - 1:
                    # exp + all-gather this half of the bias as soon as its
                    # rows are done, overlapping the rest of phase 1.
                    lo = 0 if b == HB - 1 else HB
                    sl = bloc[:, :, :, lo:lo + HB]
                    nc.scalar.activation(sl, sl, AF.Exp)
                    bl_d = dp.tile([C, T, H, HB], bf16, tag=f"bld{lo}")
                    nc.sync.dma_start(bl_d[:], sl)
                    bg_d = dp.tile([NCORES, C, T, H, HB], bf16, tag=f"bgd{lo}",
                                   addr_space="Shared")
                    nc.gpsimd.collective_compute(
                        "AllGather", mybir.AluOpType.bypass,
                        replica_groups=[list(range(NCORES))],
                        ins=[bl_d[:].opt()], outs=[bg_d[:].opt()])
                    bg_ds.append((lo, bg_d))

            eb = cp.tile([C, T, H, NCORES, BL], bf16, tag="eb")
            for lo, bg_d in bg_ds:
                for t in range(T):
                    for h in range(H):
                        nc.gpsimd.dma_start(
                            eb[:, t, h, :, lo:lo + HB],
                            bg_d[:, :, t, h, :].rearrange("c j b -> j c b"))

            # ---------------- phase 2: software-pipelined row loop ----------
            proj_tiles = {}
            acc_tiles = {}

            def proj(b):
                qp = pst.tile([C, N], f32, tag="tmp")
                nc.tensor.matmul(qp[:], wq_s[:], xT[:, b, :], start=True, stop=True)
                qT = pj.tile([C, N], bf16, tag="q")
                nc.scalar.copy(qT[:], qp[:])
                kp = pst.tile([C, N], f32, tag="tmp")
                nc.tensor.matmul(kp[:], wk_s[:], xT[:, b, :], start=True, stop=True)
                kT = pj.tile([C, N], bf16, tag="k")
                nc.vector.tensor_copy(kT[:], kp[:])
                gp = pst.tile([C, N], f32, tag="tmp")
                nc.tensor.matmul(gp[:], wg_s[:], xT[:, b, :], start=True, stop=True)
                tg = pj.tile([C, N], bf16, tag="g")
                nc.scalar.activation(tg[:], gp[:], AF.Tanh, scale=0.5)
                vp = pst.tile([C, T, C], f32, tag="tmp")
                for t in range(T):
                    nc.tensor.matmul(vp[:, t], xT[:, b, t * C:(t + 1) * C],
                                     wv_s[:], start=True, stop=True,
                                     skip_group_check=True)
                v = pj.tile([C, T, C], bf16, tag="v")
                nc.vector.tensor_copy(
                    v[:].rearrange("p t c -> p (t c)"),
                    vp[:].rearrange("p t c -> p (t c)"))
                proj_tiles[b] = (qT, kT, tg, v)

            def attention_core(b):
                qT, kT, tg, v = proj_tiles[b]
                wa = psa.tile([C, N], f32, tag="acc")
                den = psa.tile([C, N], f32, tag="acc")
                for t in range(T):
                    for gr in range(2):
                        pl = psb.tile([C, 1024], f32, tag="big")
                        for hh in range(2):
                            h = 2 * gr + hh
                            nc.tensor.matmul(
                                pl[:, 512 * hh:512 * hh + N],
                                kT[32 * h:32 * h + 32, t * C:(t + 1) * C],
                                qT[32 * h:32 * h + 32, :],
                                start=True, stop=True, tile_position=(32 * h, 0))
                        el = ap_.tile([C, 2, N], bf16, tag="el")
                        nc.scalar.activation(
                            el[:],
                            pl[:].rearrange("p (g x) -> p g x", g=2)[:, :, 0:N],
                            AF.Exp, bias=mk_s[:, t, b:b + 1])
                        em = ap_.tile([C, 2, N], bf16, tag="em")
                        nc.vector.tensor_mul(em[:], el[:],
                                             eb[:, t, 2 * gr:2 * gr + 2]
                                             .rearrange("p h c b -> p h (c b)"))
                        for hh in range(2):
                            h = 2 * gr + hh
                            nc.tensor.matmul(
                                wa[32 * h:32 * h + 32, :],
                                v[:, t, 32 * h:32 * h + 32],
                                em[:, hh, :], start=(t == 0), stop=(t == T - 1),
                                tile_position=(0, 32 * h), skip_group_check=True)
                            nc.tensor.matmul(
                                den[32 * h:32 * h + 32, :], on_s[:],
                                em[:, hh, :], start=(t == 0), stop=(t == T - 1),
                                tile_position=(0, 32 * h), skip_group_check=True)
                acc_tiles[b] = (wa, den)

            def epilogue(b):
                qT, kT, tg, v = proj_tiles.pop(b)
                wa, den = acc_tiles.pop(b)
                rec = epi.tile([C, N], f32, tag="rec")
                nc.vector.reciprocal(rec[:], den[:])
                wan = epi.tile([C, N], bf16, tag="wan")
                nc.vector.tensor_mul(wan[:], wa[:], rec[:])
                go = epi.tile([C, N], bf16, tag="go")
                nc.vector.scalar_tensor_tensor(
                    go[:], tg[:], 1.0, wan[:], ALU.add, ALU.mult)
                o_ps = pst.tile([C, N], f32, tag="tmp")
                for t in range(T):
                    nc.tensor.matmul(o_ps[:, t * C:(t + 1) * C],
                                     go[:, t * C:(t + 1) * C], wo_s[:],
                                     start=True, stop=True, skip_group_check=True)
                o = epi.tile([C, T, C], f32, tag="o")
                nc.vector.tensor_copy(o[:], o_ps[:].rearrange("p (t c) -> p t c", t=T))
                nc.sync.dma_start(
                    out_p[b].rearrange("(t p) c -> p t c", p=C), o[:])

            for b in range(LOOK):
                proj(b)
            for b in range(BL + 1):
                if b + LOOK < BL:
                    proj(b + LOOK)
                if b < BL:
                    attention_core(b)
                if b >= 1:
                    epilogue(b - 1)

    nc.compile()
    return nc


def _get_nc():
    if "nc" not in _cache:
        _cache["nc"] = _build()
    return _cache["nc"]


def prep_in_maps(pair, mask, ln_w, ln_b, w_bias, w_q, w_k, w_v, w_g, w_o):
    pair = np.asarray(pair, dtype=np.float32)
    mask = np.asarray(mask)
    g = np.asarray(ln_w, dtype=np.float32)
    beta = np.asarray(ln_b, dtype=np.float32)
    if np.any(beta != 0):
        raise NotImplementedError("nonzero ln_b not supported")
    bf = ml_dtypes.bfloat16
    sc = 1.0 / np.sqrt(D)
    wq_t = (np.asarray(w_q) * g[None, :] * sc).T.astype(bf)
    wk_t = (np.asarray(w_k) * g[None, :]).T.astype(bf)
    wv_t = (np.asarray(w_v) * g[None, :]).T.astype(bf)
    wg_t = (np.asarray(w_g) * g[None, :]).T.astype(bf)
    wb_t = (np.asarray(w_bias) * g[None, :]).T.astype(bf)
    wo_t = (np.asarray(w_o).T * 0.5).astype(np.float32).astype(bf)
    maskb = np.where(mask, 0.0, -1e9).astype(np.float32)
    id128 = np.eye(C, dtype=bf)
    ones32 = np.ones((C, D), dtype=bf)

    in_maps = []
    for c in range(NCORES):
        sl = slice(c * BL, (c + 1) * BL)
        in_maps.append({
            "pair_l": np.ascontiguousarray(pair[sl].astype(bf)),
            "maskt_l": np.ascontiguousarray(maskb[sl].T),
            "wq": wq_t, "wk": wk_t, "wv": wv_t, "wg": wg_t,
            "wo": wo_t, "wb": wb_t, "id128": id128, "ones32": ones32,
        })
    return in_maps


def kernel(pair, mask, ln_w, ln_b, w_bias, w_q, w_k, w_v, w_g, w_o):
    from concourse.bass_utils import run_bass_kernel_spmd

    in_maps = prep_in_maps(pair, mask, ln_w, ln_b, w_bias, w_q, w_k, w_v,
                           w_g, w_o)
    nc = _get_nc()
    kernel.last_in_maps = in_maps
    res = run_bass_kernel_spmd(nc, in_maps, core_ids=list(range(NCORES)))
    out = np.empty((N, N, C), dtype=np.float32)
    for c in range(NCORES):
        out[c * BL:(c + 1) * BL] = res.results[c]["out"]
    kernel.last_exec_time_ns = res.exec_time_ns
    return out
